# revision 16
# baseline (speedup 1.0000x reference)
"""BertSelfAttention (B=4, S=2048, D=1024, H=16, hd=64) on 8 trn2 NeuronCores.

Sharding: core = 2*b + half. Each core handles batch b = core//2 and 8 of the
16 heads (feature slice half*512 .. half*512+512). Fully embarrassingly
parallel: no collectives.

Per-core kernel (bf16 operands, fp32 PSUM accumulation; measured 381 us HW
exec, absmax rel err 3.3e-3 vs the fp32 reference):
  Pass A: K (f-tile 0 only), V (all), Q (f-tile 0) projections from
    X^T [1024, 2048] streamed in 512-col chunks (inputs pre-rounded to bf16
    on the host; weight f-tiles streamed per head-pair).
    Q^T, K^T in [f, s] layout (head dim on partitions); V in [s, f] layout
    with a ones column per head so the PV matmul also accumulates the
    softmax denominator in PSUM row 64.
  Attention per head-pair p, per q-quarter qq (512 wide):
    S^T chunks for both heads land in one [128, 1024] PSUM tile (head A in
    cols 0:512 via PE row-group 0-63, head B in cols 512:1024 via row-group
    64-127); one ScalarE exp per chunk with the attention mask as
    per-partition bias and the 1/sqrt(64) scale folded into the activation;
    PV accumulates ctx^T (rows 0..63) + denominator (row 64) over the 16
    k-chunks. Finalize: stage ctx out of PSUM (fast DVE copy, keeps the PE
    fed), reciprocal of the denominator row, gpsimd partition-broadcast,
    multiply, DMA out.
  K/Q projections for pair p+1 are emitted so they execute under attention
  of pair p (X^T re-streamed per pair) - keeps the PE dense so the HAM
  clock gate stays open.
"""

import numpy as np
from ml_dtypes import bfloat16 as _bf16np

S = 2048  # sequence length
DM = 1024  # model dim
F = 512  # features per core (8 heads x 64)
HL = 8  # heads per core
HD = 64  # head dim
NC = 8  # cores


def build_nc():
    import concourse.bass as bass
    import concourse.mybir as mybir
    import concourse.tile as tile
    from concourse import bacc
    from concourse.bass import ds, ts

    f32 = mybir.dt.float32
    f32r = mybir.dt.float32r
    bf16 = mybir.dt.bfloat16
    EXP = mybir.ActivationFunctionType.Exp
    PSUM = bass.MemorySpace.PSUM

    nc = bacc.Bacc("TRN2", target_bir_lowering=False, debug=False, num_devices=NC)

    x_d = nc.declare_dram_parameter("x_t", [DM, S], bf16, isOutput=False)
    wq_d = nc.declare_dram_parameter("wq_t", [DM, F], bf16, isOutput=False)
    wk_d = nc.declare_dram_parameter("wk_t", [DM, F], bf16, isOutput=False)
    wv_d = nc.declare_dram_parameter("wv_t", [DM, F], bf16, isOutput=False)
    bq_d = nc.declare_dram_parameter("bq", [F, 1], f32, isOutput=False)
    bk_d = nc.declare_dram_parameter("bk", [F, 1], f32, isOutput=False)
    bv_d = nc.declare_dram_parameter("bv", [1, F], bf16, isOutput=False)
    mask_d = nc.declare_dram_parameter("mask", [128, 16], f32, isOutput=False)
    out_d = nc.declare_dram_parameter("out_t", [F, S], f32, isOutput=True)

    mm = nc.tensor.matmul

    with tile.TileContext(nc) as tc:
        with (
            tc.tile_pool(name="const", bufs=1) as const,
            tc.tile_pool(name="w", bufs=1) as wpool,
            tc.tile_pool(name="wqk", bufs=3) as wqkp,
            tc.tile_pool(name="qkv", bufs=1) as qkv,
            tc.tile_pool(name="pqkv", bufs=2, space=PSUM) as pqkv,
            tc.tile_pool(name="s_ps", bufs=2, space=PSUM) as sp,
            tc.tile_pool(name="ctxA", bufs=1, space=PSUM) as cpA,
            tc.tile_pool(name="ctxB", bufs=1, space=PSUM) as cpB,
            tc.tile_pool(name="expp", bufs=8) as ep,
            tc.tile_pool(name="fin", bufs=3) as fp,
        ):
            # memset can't emit float32r directly; memset f32 then round-copy
            ones_f32 = const.tile([128, 128], f32)
            nc.vector.memset(ones_f32[:], 1.0)
            ones_row = const.tile([1, 128], bf16)
            nc.vector.tensor_copy(ones_row[:], ones_f32[0:1, :])
            bq_sb = const.tile([128, 4], f32)
            bk_sb = const.tile([128, 4], f32)
            for i in range(4):
                nc.sync.dma_start(bq_sb[:, i : i + 1], bq_d[ts(i, 128), :])
                nc.sync.dma_start(bk_sb[:, i : i + 1], bk_d[ts(i, 128), :])
            bv_sb = const.tile([1, F], bf16)
            nc.sync.dma_start(bv_sb[:], bv_d[:])
            mask_sb = const.tile([128, 16], f32)
            nc.sync.dma_start(mask_sb[:], mask_d[:])

            wv_sb = wpool.tile([128, 8, F], bf16)
            for c in range(8):
                nc.sync.dma_start(wv_sb[:, c, :], wv_d[ts(c, 128), :])

            def load_w_tile(w_d, i):
                wt = wqkp.tile([128, 8, 128], bf16, tag="wt")
                nc.sync.dma_start(
                    wt[:], w_d[:, ts(i, 128)].rearrange("(c p) f -> p c f", p=128)
                )
                return wt

            # X^T resident in bf16
            x_sb = qkv.tile([128, 8, S], bf16)
            for n in range(4):
                nc.sync.dma_start(
                    x_sb[:, :, ts(n, 512)],
                    x_d[:, ts(n, 512)].rearrange("(c p) s -> p c s", p=128),
                )
            # Q^T / K^T: [f, s] layout as 4 partition tiles of 128 features.
            q_sb = qkv.tile([128, 4, S], bf16)
            k_sb = qkv.tile([128, 4, S], bf16)
            # V in [k, head, d+1] layout; column 64 = 1.0 (denominator trick).
            v_sb = qkv.tile([128, 16, HL, HD + 1], bf16)
            nc.vector.tensor_copy(
                v_sb[:, :, :, HD], ones_f32[:, 0:128].rearrange("p (a b) -> p a b", a=16)
            )

            def qk_proj(wt, bsb, dst, i, n):
                ps = pqkv.tile([128, 512], f32, tag="pqkv")
                for c in range(8):
                    mm(
                        ps[:],
                        wt[:, c, :],
                        x_sb[:, c, ts(n, 512)],
                        start=(c == 0),
                        stop=(c == 7),
                    )
                nc.vector.tensor_scalar_add(
                    dst[:, i, ts(n, 512)], ps[:], bsb[:, i : i + 1]
                )

            def v_proj(m, n):
                kc = n * 4 + m
                ps = pqkv.tile([128, 512], f32, tag="pqkv")
                # bias via ones (x) bv outer product, then accumulate X@Wv^T
                mm(ps[:], ones_row[:], bv_sb[:], start=True, stop=False)
                for c in range(8):
                    mm(
                        ps[:],
                        x_sb[:, c, ds(n * 512 + m * 128, 128)],
                        wv_sb[:, c, :],
                        start=False,
                        stop=(c == 7),
                    )
                nc.vector.tensor_copy(
                    v_sb[:, kc, :, 0:HD],
                    ps[:].rearrange("p (h d) -> p h d", h=HL),
                )

            def attn_begin():
                ctxA = cpA.tile([HD + 1, 512], f32, tag="cA")
                ctxB = cpB.tile([HD + 1, 512], f32, tag="cB")
                return ctxA, ctxB

            def attn_chunk(p, qq, ctxA, ctxB, clo, chi):
                hA, hB = 2 * p, 2 * p + 1
                qsl = ds(qq * 512, 512)
                for c in range(clo, chi):
                    sps = sp.tile([128, 1024], f32, tag="s")
                    mm(
                        sps[:, 0:512],
                        k_sb[0:64, p, ds(c * 128, 128)],
                        q_sb[0:64, p, qsl],
                        start=True,
                        stop=True,
                        tile_position=(0, 0),
                    )
                    mm(
                        sps[:, 512:1024],
                        k_sb[64:128, p, ds(c * 128, 128)],
                        q_sb[64:128, p, qsl],
                        start=True,
                        stop=True,
                        tile_position=(64, 0),
                    )
                    et = ep.tile([128, 1024], bf16, tag="e")
                    nc.scalar.activation(
                        et[:], sps[:], EXP, bias=mask_sb[:, c : c + 1], scale=0.125
                    )
                    mm(
                        ctxA[:],
                        v_sb[:, c, hA, :],
                        et[:, 0:512],
                        start=(c == 0),
                        stop=(c == 15),
                    )
                    mm(
                        ctxB[:],
                        v_sb[:, c, hB, :],
                        et[:, 512:1024],
                        start=(c == 0),
                        stop=(c == 15),
                    )
            def attn_fin(p, qq, ctxA, ctxB):
                hA, hB = 2 * p, 2 * p + 1
                qsl = ds(qq * 512, 512)
                for h, ctx in ((hA, ctxA), (hB, ctxB)):
                    # stage out of PSUM fast so the next qq's PV can start;
                    # the normalize chain then runs off the critical path
                    stage = fp.tile([HD + 1, 512], f32, tag="stage")
                    nc.vector.tensor_copy(stage[:], ctx[:])
                    recip = fp.tile([1, 512], f32, tag="recip")
                    nc.vector.reciprocal(recip[:], stage[HD : HD + 1, :])
                    bcast = fp.tile([64, 512], f32, tag="bcast")
                    nc.gpsimd.partition_broadcast(bcast[:], recip[:])
                    out_sb = fp.tile([64, 512], f32, tag="out")
                    nc.vector.tensor_mul(out_sb[:], stage[0:HD, :], bcast[:])
                    nc.sync.dma_start(out_d[ds(h * 64, 64), qsl], out_sb[:])

            # ---- pass A: K(i=0), V(all), Q(j=0); attention (p0, qq0)
            # chunks ride along as their K/V chunks land ----
            wkt = load_w_tile(wk_d, 0)
            wqt = load_w_tile(wq_d, 0)
            ctx0 = None
            for n in range(4):
                qk_proj(wkt, bk_sb, k_sb, 0, n)
                for m in range(4):
                    v_proj(m, n)
                qk_proj(wqt, bq_sb, q_sb, 0, n)
                if n == 0:
                    ctx0 = attn_begin()
                attn_chunk(0, 0, ctx0[0], ctx0[1], 4 * n, 4 * n + 4)
            attn_fin(0, 0, ctx0[0], ctx0[1])

            # ---- attention pair p overlapped with projections for p+1 ----
            for p in range(4):
                if p > 0:
                    wkt = load_w_tile(wk_d, p)
                    wqt = load_w_tile(wq_d, p)
                    for n in range(4):
                        qk_proj(wkt, bk_sb, k_sb, p, n)
                        qk_proj(wqt, bq_sb, q_sb, p, n)
                for qq in range(4):
                    if p == 0 and qq == 0:
                        continue
                    ctxA, ctxB = attn_begin()
                    attn_chunk(p, qq, ctxA, ctxB, 0, 16)
                    attn_fin(p, qq, ctxA, ctxB)

    nc.compile()
    return nc


def make_in_maps(
    hidden_states, attention_mask, q_weight, q_bias, k_weight, k_bias, v_weight, v_bias
):
    hs = np.asarray(hidden_states, dtype=np.float32)
    am = np.asarray(attention_mask, dtype=np.float32)
    ws = {
        "q": np.asarray(q_weight, dtype=np.float32),
        "k": np.asarray(k_weight, dtype=np.float32),
        "v": np.asarray(v_weight, dtype=np.float32),
    }
    bs = {
        "q": np.asarray(q_bias, dtype=np.float32),
        "k": np.asarray(k_bias, dtype=np.float32),
        "v": np.asarray(v_bias, dtype=np.float32),
    }
    in_maps = []
    for core in range(NC):
        b, half = divmod(core, 2)
        fsl = slice(half * F, (half + 1) * F)
        in_maps.append(
            {
                "x_t": np.ascontiguousarray(hs[b].T).astype(_bf16np),
                "wq_t": np.ascontiguousarray(ws["q"][fsl, :].T).astype(_bf16np),
                "wk_t": np.ascontiguousarray(ws["k"][fsl, :].T).astype(_bf16np),
                "wv_t": np.ascontiguousarray(ws["v"][fsl, :].T).astype(_bf16np),
                "bq": np.ascontiguousarray(bs["q"][fsl]).reshape(F, 1),
                "bk": np.ascontiguousarray(bs["k"][fsl]).reshape(F, 1),
                "bv": np.ascontiguousarray(bs["v"][fsl]).reshape(1, F).astype(_bf16np),
                "mask": np.ascontiguousarray(am[b, 0, 0, :].reshape(16, 128).T),
            }
        )
    return in_maps


def assemble_out(results):
    out = np.empty((4, S, DM), dtype=np.float32)
    for core in range(NC):
        b, half = divmod(core, 2)
        out[b, :, half * F : (half + 1) * F] = results[core]["out_t"].T
    return out


_NC_CACHE = []


def _run(inputs, trace=False):
    from concourse.bass_utils import run_bass_kernel_spmd

    if not _NC_CACHE:
        _NC_CACHE.append(build_nc())
    nc = _NC_CACHE[0]
    in_maps = make_in_maps(**inputs)
    res = run_bass_kernel_spmd(nc, in_maps, list(range(NC)), trace=trace)
    return assemble_out(res.results), res


def kernel(**inputs):
    out, _ = _run(inputs, trace=False)
    return out


# revision 17
# speedup vs baseline: 1.0427x; 1.0427x over previous
"""BertSelfAttention (B=4, S=2048, D=1024, H=16, hd=64) on 8 trn2 NeuronCores.

Sharding: core = 2*b + half. Each core handles batch b = core//2 and 8 of the
16 heads (feature slice half*512 .. half*512+512). Fully embarrassingly
parallel: no collectives.

Per-core kernel (bf16 operands, fp32 PSUM accumulation; measured 381 us HW
exec, absmax rel err 3.3e-3 vs the fp32 reference):
  Pass A: K (f-tile 0 only), V (all), Q (f-tile 0) projections from
    X^T [1024, 2048] streamed in 512-col chunks (inputs pre-rounded to bf16
    on the host; weight f-tiles streamed per head-pair).
    Q^T, K^T in [f, s] layout (head dim on partitions); V in [s, f] layout
    with a ones column per head so the PV matmul also accumulates the
    softmax denominator in PSUM row 64.
  Attention per head-pair p, per q-quarter qq (512 wide):
    S^T chunks for both heads land in one [128, 1024] PSUM tile (head A in
    cols 0:512 via PE row-group 0-63, head B in cols 512:1024 via row-group
    64-127); one ScalarE exp per chunk with the attention mask as
    per-partition bias and the 1/sqrt(64) scale folded into the activation;
    PV accumulates ctx^T (rows 0..63) + denominator (row 64) over the 16
    k-chunks. Finalize: stage ctx out of PSUM (fast DVE copy, keeps the PE
    fed), reciprocal of the denominator row, gpsimd partition-broadcast,
    multiply, DMA out.
  K/Q projections for pair p+1 are emitted so they execute under attention
  of pair p (X^T re-streamed per pair) - keeps the PE dense so the HAM
  clock gate stays open.
"""

import numpy as np
from ml_dtypes import bfloat16 as _bf16np

S = 2048  # sequence length
DM = 1024  # model dim
F = 512  # features per core (8 heads x 64)
HL = 8  # heads per core
HD = 64  # head dim
NC = 8  # cores


def build_nc():
    import concourse.bass as bass
    import concourse.mybir as mybir
    import concourse.tile as tile
    from concourse import bacc
    from concourse.bass import ds, ts

    f32 = mybir.dt.float32
    f32r = mybir.dt.float32r
    bf16 = mybir.dt.bfloat16
    EXP = mybir.ActivationFunctionType.Exp
    PSUM = bass.MemorySpace.PSUM

    nc = bacc.Bacc("TRN2", target_bir_lowering=False, debug=False, num_devices=NC)

    x_d = nc.declare_dram_parameter("x_t", [DM, S], bf16, isOutput=False)
    wq_d = nc.declare_dram_parameter("wq_t", [DM, F], bf16, isOutput=False)
    wk_d = nc.declare_dram_parameter("wk_t", [DM, F], bf16, isOutput=False)
    wv_d = nc.declare_dram_parameter("wv_t", [DM, F], bf16, isOutput=False)
    bq_d = nc.declare_dram_parameter("bq", [F, 1], f32, isOutput=False)
    bk_d = nc.declare_dram_parameter("bk", [F, 1], f32, isOutput=False)
    bv_d = nc.declare_dram_parameter("bv", [1, F], bf16, isOutput=False)
    mask_d = nc.declare_dram_parameter("mask", [128, 16], f32, isOutput=False)
    out_d = nc.declare_dram_parameter("out_t", [F, S], f32, isOutput=True)

    mm = nc.tensor.matmul

    with tile.TileContext(nc) as tc:
        with (
            tc.tile_pool(name="const", bufs=1) as const,
            tc.tile_pool(name="w", bufs=1) as wpool,
            tc.tile_pool(name="wqk", bufs=3) as wqkp,
            tc.tile_pool(name="qkv", bufs=1) as qkv,
            tc.tile_pool(name="x", bufs=3) as xpool,
            tc.tile_pool(name="pqkv", bufs=2, space=PSUM) as pqkv,
            tc.tile_pool(name="s_ps", bufs=2, space=PSUM) as sp,
            tc.tile_pool(name="ctxA", bufs=1, space=PSUM) as cpA,
            tc.tile_pool(name="ctxB", bufs=1, space=PSUM) as cpB,
            tc.tile_pool(name="expp", bufs=8) as ep,
            tc.tile_pool(name="fin", bufs=3) as fp,
        ):
            # memset can't emit float32r directly; memset f32 then round-copy
            ones_f32 = const.tile([128, 128], f32)
            nc.vector.memset(ones_f32[:], 1.0)
            ones_row = const.tile([1, 128], bf16)
            nc.vector.tensor_copy(ones_row[:], ones_f32[0:1, :])
            bq_sb = const.tile([128, 4], f32)
            bk_sb = const.tile([128, 4], f32)
            for i in range(4):
                nc.sync.dma_start(bq_sb[:, i : i + 1], bq_d[ts(i, 128), :])
                nc.sync.dma_start(bk_sb[:, i : i + 1], bk_d[ts(i, 128), :])
            bv_sb = const.tile([1, F], bf16)
            nc.sync.dma_start(bv_sb[:], bv_d[:])
            mask_sb = const.tile([128, 16], f32)
            nc.sync.dma_start(mask_sb[:], mask_d[:])

            wv_sb = wpool.tile([128, 8, F], bf16)
            for c in range(8):
                nc.sync.dma_start(wv_sb[:, c, :], wv_d[ts(c, 128), :])

            def load_w_tile(w_d, i):
                wt = wqkp.tile([128, 8, 128], bf16, tag="wt")
                nc.sync.dma_start(
                    wt[:], w_d[:, ts(i, 128)].rearrange("(c p) f -> p c f", p=128)
                )
                return wt

            # Q^T / K^T: [f, s] layout as 4 partition tiles of 128 features.
            q_sb = qkv.tile([128, 4, S], bf16)
            k_sb = qkv.tile([128, 4, S], bf16)
            # V in [k, head, d+1] layout; column 64 = 1.0 (denominator trick).
            v_sb = qkv.tile([128, 16, HL, HD + 1], bf16)
            nc.vector.tensor_copy(
                v_sb[:, :, :, HD], ones_f32[:, 0:128].rearrange("p (a b) -> p a b", a=16)
            )

            def qk_proj(wt, bsb, dst, i, n, x_n):
                ps = pqkv.tile([128, 512], f32, tag="pqkv")
                for c in range(8):
                    mm(
                        ps[:],
                        wt[:, c, :],
                        x_n[:, c, :],
                        start=(c == 0),
                        stop=(c == 7),
                    )
                nc.vector.tensor_scalar_add(
                    dst[:, i, ts(n, 512)], ps[:], bsb[:, i : i + 1]
                )

            def v_proj(m, n, x_n):
                kc = n * 4 + m
                ps = pqkv.tile([128, 512], f32, tag="pqkv")
                # bias via ones (x) bv outer product, then accumulate X@Wv^T
                mm(ps[:], ones_row[:], bv_sb[:], start=True, stop=False)
                for c in range(8):
                    mm(
                        ps[:],
                        x_n[:, c, ts(m, 128)],
                        wv_sb[:, c, :],
                        start=False,
                        stop=(c == 7),
                    )
                nc.vector.tensor_copy(
                    v_sb[:, kc, :, 0:HD],
                    ps[:].rearrange("p (h d) -> p h d", h=HL),
                )

            def attn_pair(p, qq):
                hA, hB = 2 * p, 2 * p + 1
                qsl = ds(qq * 512, 512)
                ctxA = cpA.tile([HD + 1, 512], f32, tag="cA")
                ctxB = cpB.tile([HD + 1, 512], f32, tag="cB")
                for c in range(16):
                    sps = sp.tile([128, 1024], f32, tag="s")
                    mm(
                        sps[:, 0:512],
                        k_sb[0:64, p, ds(c * 128, 128)],
                        q_sb[0:64, p, qsl],
                        start=True,
                        stop=True,
                        tile_position=(0, 0),
                    )
                    mm(
                        sps[:, 512:1024],
                        k_sb[64:128, p, ds(c * 128, 128)],
                        q_sb[64:128, p, qsl],
                        start=True,
                        stop=True,
                        tile_position=(64, 0),
                    )
                    et = ep.tile([128, 1024], bf16, tag="e")
                    nc.scalar.activation(
                        et[:], sps[:], EXP, bias=mask_sb[:, c : c + 1], scale=0.125
                    )
                    mm(
                        ctxA[:],
                        v_sb[:, c, hA, :],
                        et[:, 0:512],
                        start=(c == 0),
                        stop=(c == 15),
                    )
                    mm(
                        ctxB[:],
                        v_sb[:, c, hB, :],
                        et[:, 512:1024],
                        start=(c == 0),
                        stop=(c == 15),
                    )
                for h, ctx in ((hA, ctxA), (hB, ctxB)):
                    # stage out of PSUM fast so the next qq's PV can start;
                    # the normalize chain then runs off the critical path
                    stage = fp.tile([HD + 1, 512], f32, tag="stage")
                    nc.vector.tensor_copy(stage[:], ctx[:])
                    recip = fp.tile([1, 512], f32, tag="recip")
                    nc.vector.reciprocal(recip[:], stage[HD : HD + 1, :])
                    bcast = fp.tile([64, 512], f32, tag="bcast")
                    nc.gpsimd.partition_broadcast(bcast[:], recip[:])
                    out_sb = fp.tile([64, 512], f32, tag="out")
                    nc.vector.tensor_mul(out_sb[:], stage[0:HD, :], bcast[:])
                    nc.sync.dma_start(out_d[ds(h * 64, 64), qsl], out_sb[:])

            # ---- pass A: K(i=0), V(all), Q(j=0), streaming X^T ----
            wkt = load_w_tile(wk_d, 0)
            wqt = load_w_tile(wq_d, 0)
            for n in range(4):
                x_n = xpool.tile([128, 8, 512], bf16, tag="x")
                nc.sync.dma_start(
                    x_n[:], x_d[:, ts(n, 512)].rearrange("(c p) s -> p c s", p=128)
                )
                qk_proj(wkt, bk_sb, k_sb, 0, n, x_n)
                for m in range(4):
                    v_proj(m, n, x_n)
                qk_proj(wqt, bq_sb, q_sb, 0, n, x_n)

            # ---- attention pair p overlapped with projections for p+1 ----
            for p in range(4):
                if p > 0:
                    wkt = load_w_tile(wk_d, p)
                    wqt = load_w_tile(wq_d, p)
                    for n in range(4):
                        x_n = xpool.tile([128, 8, 512], bf16, tag="x")
                        nc.sync.dma_start(
                            x_n[:],
                            x_d[:, ts(n, 512)].rearrange("(c p) s -> p c s", p=128),
                        )
                        qk_proj(wkt, bk_sb, k_sb, p, n, x_n)
                        qk_proj(wqt, bq_sb, q_sb, p, n, x_n)
                for qq in range(4):
                    attn_pair(p, qq)

    nc.compile()
    return nc


def make_in_maps(
    hidden_states, attention_mask, q_weight, q_bias, k_weight, k_bias, v_weight, v_bias
):
    hs = np.asarray(hidden_states, dtype=np.float32)
    am = np.asarray(attention_mask, dtype=np.float32)
    ws = {
        "q": np.asarray(q_weight, dtype=np.float32),
        "k": np.asarray(k_weight, dtype=np.float32),
        "v": np.asarray(v_weight, dtype=np.float32),
    }
    bs = {
        "q": np.asarray(q_bias, dtype=np.float32),
        "k": np.asarray(k_bias, dtype=np.float32),
        "v": np.asarray(v_bias, dtype=np.float32),
    }
    in_maps = []
    for core in range(NC):
        b, half = divmod(core, 2)
        fsl = slice(half * F, (half + 1) * F)
        in_maps.append(
            {
                "x_t": np.ascontiguousarray(hs[b].T).astype(_bf16np),
                "wq_t": np.ascontiguousarray(ws["q"][fsl, :].T).astype(_bf16np),
                "wk_t": np.ascontiguousarray(ws["k"][fsl, :].T).astype(_bf16np),
                "wv_t": np.ascontiguousarray(ws["v"][fsl, :].T).astype(_bf16np),
                "bq": np.ascontiguousarray(bs["q"][fsl]).reshape(F, 1),
                "bk": np.ascontiguousarray(bs["k"][fsl]).reshape(F, 1),
                "bv": np.ascontiguousarray(bs["v"][fsl]).reshape(1, F).astype(_bf16np),
                "mask": np.ascontiguousarray(am[b, 0, 0, :].reshape(16, 128).T),
            }
        )
    return in_maps


def assemble_out(results):
    out = np.empty((4, S, DM), dtype=np.float32)
    for core in range(NC):
        b, half = divmod(core, 2)
        out[b, :, half * F : (half + 1) * F] = results[core]["out_t"].T
    return out


_NC_CACHE = []


def _run(inputs, trace=False):
    from concourse.bass_utils import run_bass_kernel_spmd

    if not _NC_CACHE:
        _NC_CACHE.append(build_nc())
    nc = _NC_CACHE[0]
    in_maps = make_in_maps(**inputs)
    res = run_bass_kernel_spmd(nc, in_maps, list(range(NC)), trace=trace)
    return assemble_out(res.results), res


def kernel(**inputs):
    out, _ = _run(inputs, trace=False)
    return out


# revision 18
# speedup vs baseline: 1.0546x; 1.0115x over previous
"""BertSelfAttention (B=4, S=2048, D=1024, H=16, hd=64) on 8 trn2 NeuronCores.

Sharding: core = 2*b + half. Each core handles batch b = core//2 and 8 of the
16 heads (feature slice half*512 .. half*512+512). Fully embarrassingly
parallel: no collectives.

Per-core kernel (bf16 operands, fp32 PSUM accumulation; measured 381 us HW
exec, absmax rel err 3.3e-3 vs the fp32 reference):
  Pass A: K (f-tile 0 only), V (all), Q (f-tile 0) projections from
    X^T [1024, 2048] streamed in 512-col chunks (inputs pre-rounded to bf16
    on the host; weight f-tiles streamed per head-pair).
    Q^T, K^T in [f, s] layout (head dim on partitions); V in [s, f] layout
    with a ones column per head so the PV matmul also accumulates the
    softmax denominator in PSUM row 64.
  Attention per head-pair p, per q-quarter qq (512 wide):
    S^T chunks for both heads land in one [128, 1024] PSUM tile (head A in
    cols 0:512 via PE row-group 0-63, head B in cols 512:1024 via row-group
    64-127); one ScalarE exp per chunk with the attention mask as
    per-partition bias and the 1/sqrt(64) scale folded into the activation;
    PV accumulates ctx^T (rows 0..63) + denominator (row 64) over the 16
    k-chunks. Finalize: stage ctx out of PSUM (fast DVE copy, keeps the PE
    fed), reciprocal of the denominator row, gpsimd partition-broadcast,
    multiply, DMA out.
  K/Q projections for pair p+1 are emitted so they execute under attention
  of pair p (X^T re-streamed per pair) - keeps the PE dense so the HAM
  clock gate stays open.
"""

import numpy as np
from ml_dtypes import bfloat16 as _bf16np

S = 2048  # sequence length
DM = 1024  # model dim
F = 512  # features per core (8 heads x 64)
HL = 8  # heads per core
HD = 64  # head dim
NC = 8  # cores


def build_nc():
    import concourse.bass as bass
    import concourse.mybir as mybir
    import concourse.tile as tile
    from concourse import bacc
    from concourse.bass import ds, ts

    f32 = mybir.dt.float32
    f32r = mybir.dt.float32r
    bf16 = mybir.dt.bfloat16
    EXP = mybir.ActivationFunctionType.Exp
    PSUM = bass.MemorySpace.PSUM

    nc = bacc.Bacc("TRN2", target_bir_lowering=False, debug=False, num_devices=NC)

    x_d = nc.declare_dram_parameter("x_t", [DM, S], bf16, isOutput=False)
    wq_d = nc.declare_dram_parameter("wq_t", [DM, F], bf16, isOutput=False)
    wk_d = nc.declare_dram_parameter("wk_t", [DM, F], bf16, isOutput=False)
    wv_d = nc.declare_dram_parameter("wv_t", [DM, F], bf16, isOutput=False)
    bq_d = nc.declare_dram_parameter("bq", [F, 1], f32, isOutput=False)
    bk_d = nc.declare_dram_parameter("bk", [F, 1], f32, isOutput=False)
    bv_d = nc.declare_dram_parameter("bv", [1, F], bf16, isOutput=False)
    mask_d = nc.declare_dram_parameter("mask", [128, 16], f32, isOutput=False)
    out_d = nc.declare_dram_parameter("out_t", [F, S], f32, isOutput=True)

    mm = nc.tensor.matmul

    with tile.TileContext(nc) as tc:
        with (
            tc.tile_pool(name="const", bufs=1) as const,
            tc.tile_pool(name="w", bufs=1) as wpool,
            tc.tile_pool(name="wqk", bufs=3) as wqkp,
            tc.tile_pool(name="qkv", bufs=1) as qkv,
            tc.tile_pool(name="x", bufs=3) as xpool,
            tc.tile_pool(name="pqkv", bufs=2, space=PSUM) as pqkv,
            tc.tile_pool(name="s_ps", bufs=2, space=PSUM) as sp,
            tc.tile_pool(name="ctxA", bufs=1, space=PSUM) as cpA,
            tc.tile_pool(name="ctxB", bufs=1, space=PSUM) as cpB,
            tc.tile_pool(name="expp", bufs=12) as ep,
            tc.tile_pool(name="fin", bufs=3) as fp,
        ):
            # memset can't emit float32r directly; memset f32 then round-copy
            ones_f32 = const.tile([128, 128], f32)
            nc.vector.memset(ones_f32[:], 1.0)
            ones_row = const.tile([1, 128], bf16)
            nc.vector.tensor_copy(ones_row[:], ones_f32[0:1, :])
            warm = const.tile([1, 1], f32)
            nc.scalar.activation(warm[:], ones_f32[0:1, 0:1], EXP)
            bq_sb = const.tile([128, 4], f32)
            bk_sb = const.tile([128, 4], f32)
            for i in range(4):
                nc.sync.dma_start(bq_sb[:, i : i + 1], bq_d[ts(i, 128), :])
                nc.sync.dma_start(bk_sb[:, i : i + 1], bk_d[ts(i, 128), :])
            bv_sb = const.tile([1, F], bf16)
            nc.sync.dma_start(bv_sb[:], bv_d[:])
            mask_sb = const.tile([128, 16], f32)
            nc.sync.dma_start(mask_sb[:], mask_d[:])

            wv_sb = wpool.tile([128, 8, F], bf16)
            for c in range(8):
                nc.sync.dma_start(wv_sb[:, c, :], wv_d[ts(c, 128), :])

            def load_w_tile(w_d, i):
                wt = wqkp.tile([128, 8, 128], bf16, tag="wt")
                nc.sync.dma_start(
                    wt[:], w_d[:, ts(i, 128)].rearrange("(c p) f -> p c f", p=128)
                )
                return wt

            # Q^T / K^T: [f, s] layout as 4 partition tiles of 128 features.
            q_sb = qkv.tile([128, 4, S], bf16)
            k_sb = qkv.tile([128, 4, S], bf16)
            # V in [k, head, d+1] layout; column 64 = 1.0 (denominator trick).
            v_sb = qkv.tile([128, 16, HL, HD + 1], bf16)
            nc.vector.tensor_copy(
                v_sb[:, :, :, HD], ones_f32[:, 0:128].rearrange("p (a b) -> p a b", a=16)
            )

            def qk_proj(wt, bsb, dst, i, n, x_n):
                ps = pqkv.tile([128, 512], f32, tag="pqkv")
                for c in range(8):
                    mm(
                        ps[:],
                        wt[:, c, :],
                        x_n[:, c, :],
                        start=(c == 0),
                        stop=(c == 7),
                    )
                nc.vector.tensor_scalar_add(
                    dst[:, i, ts(n, 512)], ps[:], bsb[:, i : i + 1]
                )

            def v_proj(m, n, x_n):
                kc = n * 4 + m
                ps = pqkv.tile([128, 512], f32, tag="pqkv")
                # bias via ones (x) bv outer product, then accumulate X@Wv^T
                mm(ps[:], ones_row[:], bv_sb[:], start=True, stop=False)
                for c in range(8):
                    mm(
                        ps[:],
                        x_n[:, c, ts(m, 128)],
                        wv_sb[:, c, :],
                        start=False,
                        stop=(c == 7),
                    )
                nc.vector.tensor_copy(
                    v_sb[:, kc, :, 0:HD],
                    ps[:].rearrange("p (h d) -> p h d", h=HL),
                )

            def attn_pair(p, qq):
                hA, hB = 2 * p, 2 * p + 1
                qsl = ds(qq * 512, 512)
                ctxA = cpA.tile([HD + 1, 512], f32, tag="cA")
                ctxB = cpB.tile([HD + 1, 512], f32, tag="cB")
                for c in range(16):
                    sps = sp.tile([128, 1024], f32, tag="s")
                    mm(
                        sps[:, 0:512],
                        k_sb[0:64, p, ds(c * 128, 128)],
                        q_sb[0:64, p, qsl],
                        start=True,
                        stop=True,
                        tile_position=(0, 0),
                    )
                    mm(
                        sps[:, 512:1024],
                        k_sb[64:128, p, ds(c * 128, 128)],
                        q_sb[64:128, p, qsl],
                        start=True,
                        stop=True,
                        tile_position=(64, 0),
                    )
                    et = ep.tile([128, 1024], bf16, tag="e")
                    nc.scalar.activation(
                        et[:], sps[:], EXP, bias=mask_sb[:, c : c + 1], scale=0.125
                    )
                    mm(
                        ctxA[:],
                        v_sb[:, c, hA, :],
                        et[:, 0:512],
                        start=(c == 0),
                        stop=(c == 15),
                    )
                    mm(
                        ctxB[:],
                        v_sb[:, c, hB, :],
                        et[:, 512:1024],
                        start=(c == 0),
                        stop=(c == 15),
                    )
                for h, ctx in ((hA, ctxA), (hB, ctxB)):
                    # stage out of PSUM fast so the next qq's PV can start;
                    # the normalize chain then runs off the critical path
                    stage = fp.tile([HD + 1, 512], f32, tag="stage")
                    nc.vector.tensor_copy(stage[:], ctx[:])
                    recip = fp.tile([1, 512], f32, tag="recip")
                    nc.vector.reciprocal(recip[:], stage[HD : HD + 1, :])
                    bcast = fp.tile([64, 512], f32, tag="bcast")
                    nc.gpsimd.partition_broadcast(bcast[:], recip[:])
                    out_sb = fp.tile([64, 512], f32, tag="out")
                    nc.vector.tensor_mul(out_sb[:], stage[0:HD, :], bcast[:])
                    nc.sync.dma_start(out_d[ds(h * 64, 64), qsl], out_sb[:])

            # ---- pass A: K(i=0), V(all), Q(j=0), streaming X^T ----
            wkt = load_w_tile(wk_d, 0)
            wqt = load_w_tile(wq_d, 0)
            for n in range(4):
                x_n = xpool.tile([128, 8, 512], bf16, tag="x")
                nc.sync.dma_start(
                    x_n[:], x_d[:, ts(n, 512)].rearrange("(c p) s -> p c s", p=128)
                )
                qk_proj(wkt, bk_sb, k_sb, 0, n, x_n)
                for m in range(4):
                    v_proj(m, n, x_n)
                qk_proj(wqt, bq_sb, q_sb, 0, n, x_n)

            # ---- attention pair p overlapped with projections for p+1 ----
            for p in range(4):
                if p > 0:
                    wkt = load_w_tile(wk_d, p)
                    wqt = load_w_tile(wq_d, p)
                    for n in range(4):
                        x_n = xpool.tile([128, 8, 512], bf16, tag="x")
                        nc.sync.dma_start(
                            x_n[:],
                            x_d[:, ts(n, 512)].rearrange("(c p) s -> p c s", p=128),
                        )
                        qk_proj(wkt, bk_sb, k_sb, p, n, x_n)
                        qk_proj(wqt, bq_sb, q_sb, p, n, x_n)
                for qq in range(4):
                    attn_pair(p, qq)

    nc.compile()
    return nc


def make_in_maps(
    hidden_states, attention_mask, q_weight, q_bias, k_weight, k_bias, v_weight, v_bias
):
    hs = np.asarray(hidden_states, dtype=np.float32)
    am = np.asarray(attention_mask, dtype=np.float32)
    ws = {
        "q": np.asarray(q_weight, dtype=np.float32),
        "k": np.asarray(k_weight, dtype=np.float32),
        "v": np.asarray(v_weight, dtype=np.float32),
    }
    bs = {
        "q": np.asarray(q_bias, dtype=np.float32),
        "k": np.asarray(k_bias, dtype=np.float32),
        "v": np.asarray(v_bias, dtype=np.float32),
    }
    in_maps = []
    for core in range(NC):
        b, half = divmod(core, 2)
        fsl = slice(half * F, (half + 1) * F)
        in_maps.append(
            {
                "x_t": np.ascontiguousarray(hs[b].T).astype(_bf16np),
                "wq_t": np.ascontiguousarray(ws["q"][fsl, :].T).astype(_bf16np),
                "wk_t": np.ascontiguousarray(ws["k"][fsl, :].T).astype(_bf16np),
                "wv_t": np.ascontiguousarray(ws["v"][fsl, :].T).astype(_bf16np),
                "bq": np.ascontiguousarray(bs["q"][fsl]).reshape(F, 1),
                "bk": np.ascontiguousarray(bs["k"][fsl]).reshape(F, 1),
                "bv": np.ascontiguousarray(bs["v"][fsl]).reshape(1, F).astype(_bf16np),
                "mask": np.ascontiguousarray(am[b, 0, 0, :].reshape(16, 128).T),
            }
        )
    return in_maps


def assemble_out(results):
    out = np.empty((4, S, DM), dtype=np.float32)
    for core in range(NC):
        b, half = divmod(core, 2)
        out[b, :, half * F : (half + 1) * F] = results[core]["out_t"].T
    return out


_NC_CACHE = []


def _run(inputs, trace=False):
    from concourse.bass_utils import run_bass_kernel_spmd

    if not _NC_CACHE:
        _NC_CACHE.append(build_nc())
    nc = _NC_CACHE[0]
    in_maps = make_in_maps(**inputs)
    res = run_bass_kernel_spmd(nc, in_maps, list(range(NC)), trace=trace)
    return assemble_out(res.results), res


def kernel(**inputs):
    out, _ = _run(inputs, trace=False)
    return out


# revision 22
# speedup vs baseline: 1.0595x; 1.0046x over previous
"""BertSelfAttention (B=4, S=2048, D=1024, H=16, hd=64) on 8 trn2 NeuronCores.

Sharding: core = 2*b + half. Each core handles batch b = core//2 and 8 of the
16 heads (feature slice half*512 .. half*512+512). Fully embarrassingly
parallel: no collectives.

Per-core kernel (bf16 operands, fp32 PSUM accumulation; measured 381 us HW
exec, absmax rel err 3.3e-3 vs the fp32 reference):
  Pass A: K (f-tile 0 only), V (all), Q (f-tile 0) projections from
    X^T [1024, 2048] streamed in 512-col chunks (inputs pre-rounded to bf16
    on the host; weight f-tiles streamed per head-pair).
    Q^T, K^T in [f, s] layout (head dim on partitions); V in [s, f] layout
    with a ones column per head so the PV matmul also accumulates the
    softmax denominator in PSUM row 64.
  Attention per head-pair p, per q-quarter qq (512 wide):
    S^T chunks for both heads land in one [128, 1024] PSUM tile (head A in
    cols 0:512 via PE row-group 0-63, head B in cols 512:1024 via row-group
    64-127); one ScalarE exp per chunk with the attention mask as
    per-partition bias and the 1/sqrt(64) scale folded into the activation;
    PV accumulates ctx^T (rows 0..63) + denominator (row 64) over the 16
    k-chunks. Finalize: stage ctx out of PSUM (fast DVE copy, keeps the PE
    fed), reciprocal of the denominator row, gpsimd partition-broadcast,
    multiply, DMA out.
  K/Q projections for pair p+1 are emitted so they execute under attention
  of pair p (X^T re-streamed per pair) - keeps the PE dense so the HAM
  clock gate stays open.
"""

import numpy as np
from ml_dtypes import bfloat16 as _bf16np

S = 2048  # sequence length
DM = 1024  # model dim
F = 512  # features per core (8 heads x 64)
HL = 8  # heads per core
HD = 64  # head dim
NC = 8  # cores


def build_nc():
    import concourse.bass as bass
    import concourse.mybir as mybir
    import concourse.tile as tile
    from concourse import bacc
    from concourse.bass import ds, ts

    f32 = mybir.dt.float32
    f32r = mybir.dt.float32r
    bf16 = mybir.dt.bfloat16
    EXP = mybir.ActivationFunctionType.Exp
    PSUM = bass.MemorySpace.PSUM

    nc = bacc.Bacc("TRN2", target_bir_lowering=False, debug=False, num_devices=NC)

    x_d = nc.declare_dram_parameter("x_t", [4 * DM, 512], bf16, isOutput=False)
    wq_d = nc.declare_dram_parameter("wq_t", [4 * DM, 128], bf16, isOutput=False)
    wk_d = nc.declare_dram_parameter("wk_t", [4 * DM, 128], bf16, isOutput=False)
    wv_d = nc.declare_dram_parameter("wv_t", [DM, F], bf16, isOutput=False)
    bq_d = nc.declare_dram_parameter("bq", [F, 1], f32, isOutput=False)
    bk_d = nc.declare_dram_parameter("bk", [F, 1], f32, isOutput=False)
    bv_d = nc.declare_dram_parameter("bv", [1, F], bf16, isOutput=False)
    mask_d = nc.declare_dram_parameter("mask", [128, 16], f32, isOutput=False)
    out_d = nc.declare_dram_parameter("out_t", [F, S], f32, isOutput=True)

    mm = nc.tensor.matmul

    with tile.TileContext(nc) as tc:
        with (
            tc.tile_pool(name="const", bufs=1) as const,
            tc.tile_pool(name="w", bufs=1) as wpool,
            tc.tile_pool(name="wqk", bufs=3) as wqkp,
            tc.tile_pool(name="qkv", bufs=1) as qkv,
            tc.tile_pool(name="x", bufs=3) as xpool,
            tc.tile_pool(name="pqkv", bufs=2, space=PSUM) as pqkv,
            tc.tile_pool(name="s_ps", bufs=2, space=PSUM) as sp,
            tc.tile_pool(name="ctxA", bufs=1, space=PSUM) as cpA,
            tc.tile_pool(name="ctxB", bufs=1, space=PSUM) as cpB,
            tc.tile_pool(name="expp", bufs=12) as ep,
            tc.tile_pool(name="fin", bufs=3) as fp,
        ):
            # critical-path first: wk/wq f-tile 0 and x chunk 0 on the sync
            # queue before anything else touches DMA
            def load_w_tile(w_d, i):
                wt = wqkp.tile([128, 8, 128], bf16, tag="wt")
                nc.sync.dma_start(
                    wt[:],
                    w_d[ds(i * DM, DM), :].rearrange("(c p) f -> p c f", p=128),
                )
                return wt

            wkt = load_w_tile(wk_d, 0)
            wqt = load_w_tile(wq_d, 0)
            x0 = xpool.tile([128, 8, 512], bf16, tag="x")
            nc.sync.dma_start(
                x0[:], x_d[ds(0, DM), :].rearrange("(c p) s -> p c s", p=128)
            )

            # memset can't emit float32r directly; memset f32 then round-copy
            ones_f32 = const.tile([128, 128], f32)
            nc.vector.memset(ones_f32[:], 1.0)
            ones_row = const.tile([1, 128], bf16)
            nc.vector.tensor_copy(ones_row[:], ones_f32[0:1, :])
            warm = const.tile([1, 1], f32)
            nc.scalar.activation(warm[:], ones_f32[0:1, 0:1], EXP)
            bq_sb = const.tile([128, 4], f32)
            bk_sb = const.tile([128, 4], f32)
            for i in range(4):
                nc.gpsimd.dma_start(bq_sb[:, i : i + 1], bq_d[ts(i, 128), :])
                nc.gpsimd.dma_start(bk_sb[:, i : i + 1], bk_d[ts(i, 128), :])
            bv_sb = const.tile([1, F], bf16)
            nc.gpsimd.dma_start(bv_sb[:], bv_d[:])
            mask_sb = const.tile([128, 16], f32)
            nc.gpsimd.dma_start(mask_sb[:], mask_d[:])

            wv_sb = wpool.tile([128, 8, F], bf16)
            for c in range(8):
                nc.gpsimd.dma_start(wv_sb[:, c, :], wv_d[ts(c, 128), :])

            # Q^T / K^T: [f, s] layout as 4 partition tiles of 128 features.
            q_sb = qkv.tile([128, 4, S], bf16)
            k_sb = qkv.tile([128, 4, S], bf16)
            # V in [k, head, d+1] layout; column 64 = 1.0 (denominator trick).
            v_sb = qkv.tile([128, 16, HL, HD + 1], bf16)
            nc.vector.tensor_copy(
                v_sb[:, :, :, HD], ones_f32[:, 0:128].rearrange("p (a b) -> p a b", a=16)
            )

            def qk_proj(wt, bsb, dst, i, n, x_n):
                ps = pqkv.tile([128, 512], f32, tag="pqkv")
                for c in range(8):
                    mm(
                        ps[:],
                        wt[:, c, :],
                        x_n[:, c, :],
                        start=(c == 0),
                        stop=(c == 7),
                    )
                nc.vector.tensor_scalar_add(
                    dst[:, i, ts(n, 512)], ps[:], bsb[:, i : i + 1]
                )

            def v_proj(m, n, x_n):
                kc = n * 4 + m
                ps = pqkv.tile([128, 512], f32, tag="pqkv")
                # bias via ones (x) bv outer product, then accumulate X@Wv^T
                mm(ps[:], ones_row[:], bv_sb[:], start=True, stop=False)
                for c in range(8):
                    mm(
                        ps[:],
                        x_n[:, c, ts(m, 128)],
                        wv_sb[:, c, :],
                        start=False,
                        stop=(c == 7),
                    )
                nc.vector.tensor_copy(
                    v_sb[:, kc, :, 0:HD],
                    ps[:].rearrange("p (h d) -> p h d", h=HL),
                )

            def attn_pair(p, qq):
                hA, hB = 2 * p, 2 * p + 1
                qsl = ds(qq * 512, 512)
                ctxA = cpA.tile([HD + 1, 512], f32, tag="cA")
                ctxB = cpB.tile([HD + 1, 512], f32, tag="cB")
                for c in range(16):
                    sps = sp.tile([128, 1024], f32, tag="s")
                    mm(
                        sps[:, 0:512],
                        k_sb[0:64, p, ds(c * 128, 128)],
                        q_sb[0:64, p, qsl],
                        start=True,
                        stop=True,
                        tile_position=(0, 0),
                    )
                    mm(
                        sps[:, 512:1024],
                        k_sb[64:128, p, ds(c * 128, 128)],
                        q_sb[64:128, p, qsl],
                        start=True,
                        stop=True,
                        tile_position=(64, 0),
                    )
                    et = ep.tile([128, 1024], bf16, tag="e")
                    nc.scalar.activation(
                        et[:], sps[:], EXP, bias=mask_sb[:, c : c + 1], scale=0.125
                    )
                    mm(
                        ctxA[:],
                        v_sb[:, c, hA, :],
                        et[:, 0:512],
                        start=(c == 0),
                        stop=(c == 15),
                    )
                    mm(
                        ctxB[:],
                        v_sb[:, c, hB, :],
                        et[:, 512:1024],
                        start=(c == 0),
                        stop=(c == 15),
                    )
                for h, ctx in ((hA, ctxA), (hB, ctxB)):
                    # stage out of PSUM fast so the next qq's PV can start;
                    # the normalize chain then runs off the critical path
                    stage = fp.tile([HD + 1, 512], f32, tag="stage")
                    nc.vector.tensor_copy(stage[:], ctx[:])
                    recip = fp.tile([1, 512], f32, tag="recip")
                    nc.vector.reciprocal(recip[:], stage[HD : HD + 1, :])
                    bcast = fp.tile([64, 512], f32, tag="bcast")
                    nc.gpsimd.partition_broadcast(bcast[:], recip[:])
                    out_sb = fp.tile([64, 512], f32, tag="out")
                    nc.vector.tensor_mul(out_sb[:], stage[0:HD, :], bcast[:])
                    nc.sync.dma_start(out_d[ds(h * 64, 64), qsl], out_sb[:])

            # ---- pass A: K(i=0), V(all), Q(j=0), streaming X^T ----
            for n in range(4):
                if n == 0:
                    x_n = x0
                else:
                    x_n = xpool.tile([128, 8, 512], bf16, tag="x")
                    nc.sync.dma_start(
                        x_n[:],
                        x_d[ds(n * DM, DM), :].rearrange("(c p) s -> p c s", p=128),
                    )
                qk_proj(wkt, bk_sb, k_sb, 0, n, x_n)
                for m in range(4):
                    v_proj(m, n, x_n)
                qk_proj(wqt, bq_sb, q_sb, 0, n, x_n)

            # ---- attention pair p overlapped with projections for p+1 ----
            for p in range(4):
                if p > 0:
                    wkt = load_w_tile(wk_d, p)
                    wqt = load_w_tile(wq_d, p)
                    for n in range(4):
                        x_n = xpool.tile([128, 8, 512], bf16, tag="x")
                        nc.sync.dma_start(
                            x_n[:],
                            x_d[ds(n * DM, DM), :].rearrange("(c p) s -> p c s", p=128),
                        )
                        qk_proj(wkt, bk_sb, k_sb, p, n, x_n)
                        qk_proj(wqt, bq_sb, q_sb, p, n, x_n)
                for qq in range(4):
                    attn_pair(p, qq)

    nc.compile()
    return nc


def make_in_maps(
    hidden_states, attention_mask, q_weight, q_bias, k_weight, k_bias, v_weight, v_bias
):
    hs = np.asarray(hidden_states, dtype=np.float32)
    am = np.asarray(attention_mask, dtype=np.float32)
    ws = {
        "q": np.asarray(q_weight, dtype=np.float32),
        "k": np.asarray(k_weight, dtype=np.float32),
        "v": np.asarray(v_weight, dtype=np.float32),
    }
    bs = {
        "q": np.asarray(q_bias, dtype=np.float32),
        "k": np.asarray(k_bias, dtype=np.float32),
        "v": np.asarray(v_bias, dtype=np.float32),
    }
    in_maps = []
    for core in range(NC):
        b, half = divmod(core, 2)
        fsl = slice(half * F, (half + 1) * F)
        in_maps.append(
            {
                "x_t": np.ascontiguousarray(
                    hs[b].T.reshape(DM, 4, 512).transpose(1, 0, 2).reshape(4 * DM, 512)
                ).astype(_bf16np),
                "wq_t": np.ascontiguousarray(
                    ws["q"][fsl, :].T.reshape(DM, 4, 128).transpose(1, 0, 2).reshape(4 * DM, 128)
                ).astype(_bf16np),
                "wk_t": np.ascontiguousarray(
                    ws["k"][fsl, :].T.reshape(DM, 4, 128).transpose(1, 0, 2).reshape(4 * DM, 128)
                ).astype(_bf16np),
                "wv_t": np.ascontiguousarray(ws["v"][fsl, :].T).astype(_bf16np),
                "bq": np.ascontiguousarray(bs["q"][fsl]).reshape(F, 1),
                "bk": np.ascontiguousarray(bs["k"][fsl]).reshape(F, 1),
                "bv": np.ascontiguousarray(bs["v"][fsl]).reshape(1, F).astype(_bf16np),
                "mask": np.ascontiguousarray(am[b, 0, 0, :].reshape(16, 128).T),
            }
        )
    return in_maps


def assemble_out(results):
    out = np.empty((4, S, DM), dtype=np.float32)
    for core in range(NC):
        b, half = divmod(core, 2)
        out[b, :, half * F : (half + 1) * F] = results[core]["out_t"].T
    return out


_NC_CACHE = []


def _run(inputs, trace=False):
    from concourse.bass_utils import run_bass_kernel_spmd

    if not _NC_CACHE:
        _NC_CACHE.append(build_nc())
    nc = _NC_CACHE[0]
    in_maps = make_in_maps(**inputs)
    res = run_bass_kernel_spmd(nc, in_maps, list(range(NC)), trace=trace)
    return assemble_out(res.results), res


def kernel(**inputs):
    out, _ = _run(inputs, trace=False)
    return out


# revision 23
# speedup vs baseline: 1.0780x; 1.0174x over previous
"""BertSelfAttention (B=4, S=2048, D=1024, H=16, hd=64) on 8 trn2 NeuronCores.

Sharding: core = 2*b + half. Each core handles batch b = core//2 and 8 of the
16 heads (feature slice half*512 .. half*512+512). Fully embarrassingly
parallel: no collectives.

Per-core kernel (bf16 operands, fp32 PSUM accumulation; measured 381 us HW
exec, absmax rel err 3.3e-3 vs the fp32 reference):
  Pass A: K (f-tile 0 only), V (all), Q (f-tile 0) projections from
    X^T [1024, 2048] streamed in 512-col chunks (inputs pre-rounded to bf16
    on the host; weight f-tiles streamed per head-pair).
    Q^T, K^T in [f, s] layout (head dim on partitions); V in [s, f] layout
    with a ones column per head so the PV matmul also accumulates the
    softmax denominator in PSUM row 64.
  Attention per head-pair p, per q-quarter qq (512 wide):
    S^T chunks for both heads land in one [128, 1024] PSUM tile (head A in
    cols 0:512 via PE row-group 0-63, head B in cols 512:1024 via row-group
    64-127); one ScalarE exp per chunk with the attention mask as
    per-partition bias and the 1/sqrt(64) scale folded into the activation;
    PV accumulates ctx^T (rows 0..63) + denominator (row 64) over the 16
    k-chunks. Finalize: stage ctx out of PSUM (fast DVE copy, keeps the PE
    fed), reciprocal of the denominator row, gpsimd partition-broadcast,
    multiply, DMA out.
  K/Q projections for pair p+1 are emitted so they execute under attention
  of pair p (X^T re-streamed per pair) - keeps the PE dense so the HAM
  clock gate stays open.
"""

import numpy as np
from ml_dtypes import bfloat16 as _bf16np

S = 2048  # sequence length
DM = 1024  # model dim
F = 512  # features per core (8 heads x 64)
HL = 8  # heads per core
HD = 64  # head dim
NC = 8  # cores


def build_nc():
    import concourse.bass as bass
    import concourse.mybir as mybir
    import concourse.tile as tile
    from concourse import bacc
    from concourse.bass import ds, ts

    f32 = mybir.dt.float32
    f32r = mybir.dt.float32r
    bf16 = mybir.dt.bfloat16
    EXP = mybir.ActivationFunctionType.Exp
    PSUM = bass.MemorySpace.PSUM

    nc = bacc.Bacc("TRN2", target_bir_lowering=False, debug=False, num_devices=NC)

    x_d = nc.declare_dram_parameter("x_t", [4 * DM, 512], bf16, isOutput=False)
    wq_d = nc.declare_dram_parameter("wq_t", [4 * DM, 128], bf16, isOutput=False)
    wk_d = nc.declare_dram_parameter("wk_t", [4 * DM, 128], bf16, isOutput=False)
    wv_d = nc.declare_dram_parameter("wv_t", [DM, F], bf16, isOutput=False)
    bq_d = nc.declare_dram_parameter("bq", [F, 1], f32, isOutput=False)
    bk_d = nc.declare_dram_parameter("bk", [F, 1], f32, isOutput=False)
    bv_d = nc.declare_dram_parameter("bv", [F, 1], f32, isOutput=False)
    mask_d = nc.declare_dram_parameter("mask", [128, 16], f32, isOutput=False)
    out_d = nc.declare_dram_parameter("out_t", [F, S], f32, isOutput=True)

    mm = nc.tensor.matmul

    with tile.TileContext(nc) as tc:
        with (
            tc.tile_pool(name="const", bufs=1) as const,
            tc.tile_pool(name="w", bufs=1) as wpool,
            tc.tile_pool(name="wqk", bufs=3) as wqkp,
            tc.tile_pool(name="qkv", bufs=1) as qkv,
            tc.tile_pool(name="x", bufs=3) as xpool,
            tc.tile_pool(name="pqkv", bufs=2, space=PSUM) as pqkv,
            tc.tile_pool(name="s_ps", bufs=2, space=PSUM) as sp,
            tc.tile_pool(name="ctxA", bufs=1, space=PSUM) as cpA,
            tc.tile_pool(name="ctxB", bufs=1, space=PSUM) as cpB,
            tc.tile_pool(name="expp", bufs=12) as ep,
            tc.tile_pool(name="fin", bufs=3) as fp,
        ):
            # critical-path first: wk/wq f-tile 0 and x chunk 0 on the sync
            # queue before anything else touches DMA
            def load_w_tile(w_d, i):
                wt = wqkp.tile([128, 8, 128], bf16, tag="wt")
                nc.sync.dma_start(
                    wt[:],
                    w_d[ds(i * DM, DM), :].rearrange("(c p) f -> p c f", p=128),
                )
                return wt

            wkt = load_w_tile(wk_d, 0)
            wqt = load_w_tile(wq_d, 0)
            x0 = xpool.tile([128, 8, 512], bf16, tag="x")
            nc.sync.dma_start(
                x0[:], x_d[ds(0, DM), :].rearrange("(c p) s -> p c s", p=128)
            )

            # memset can't emit float32r directly; memset f32 then round-copy
            ones_f32 = const.tile([128, 128], f32)
            nc.vector.memset(ones_f32[:], 1.0)
            ones_row = const.tile([1, 128], bf16)
            nc.vector.tensor_copy(ones_row[:], ones_f32[0:1, :])
            warm = const.tile([1, 1], f32)
            nc.scalar.activation(warm[:], ones_f32[0:1, 0:1], EXP)
            bq_sb = const.tile([128, 4], f32)
            bk_sb = const.tile([128, 4], f32)
            for i in range(4):
                nc.gpsimd.dma_start(bq_sb[:, i : i + 1], bq_d[ts(i, 128), :])
                nc.gpsimd.dma_start(bk_sb[:, i : i + 1], bk_d[ts(i, 128), :])
            bv_sb = const.tile([128, 4], f32)
            for i in range(4):
                nc.gpsimd.dma_start(bv_sb[:, i : i + 1], bv_d[ts(i, 128), :])
            mask_sb = const.tile([128, 16], f32)
            nc.gpsimd.dma_start(mask_sb[:], mask_d[:])

            wv_sb = wpool.tile([128, 8, F], bf16)
            for c in range(8):
                nc.gpsimd.dma_start(wv_sb[:, c, :], wv_d[ts(c, 128), :])

            # Q^T / K^T: [f, s] layout as 4 partition tiles of 128 features.
            q_sb = qkv.tile([128, 4, S], bf16)
            k_sb = qkv.tile([128, 4, S], bf16)
            # V in [k, head, d+1] layout; column 64 = 1.0 (denominator trick).
            v_sb = qkv.tile([128, 16, HL, HD + 1], bf16)
            nc.vector.tensor_copy(
                v_sb[:, :, :, HD], ones_f32[:, 0:128].rearrange("p (a b) -> p a b", a=16)
            )

            def qk_proj(wt, bsb, dst, i, n, x_n):
                ps = pqkv.tile([128, 512], f32, tag="pqkv")
                for c in range(8):
                    mm(
                        ps[:],
                        wt[:, c, :],
                        x_n[:, c, :],
                        start=(c == 0),
                        stop=(c == 7),
                    )
                nc.vector.tensor_scalar_add(
                    dst[:, i, ts(n, 512)], ps[:], bsb[:, i : i + 1]
                )

            def v_proj(m, n, x_n):
                kc = n * 4 + m
                ps = pqkv.tile([128, 512], f32, tag="pqkv")
                for c in range(8):
                    mm(
                        ps[:],
                        x_n[:, c, ts(m, 128)],
                        wv_sb[:, c, :],
                        start=(c == 0),
                        stop=(c == 7),
                    )
                nc.vector.tensor_copy(
                    v_sb[:, kc, :, 0:HD],
                    ps[:].rearrange("p (h d) -> p h d", h=HL),
                )

            def attn_pair(p, qq):
                hA, hB = 2 * p, 2 * p + 1
                qsl = ds(qq * 512, 512)
                ctxA = cpA.tile([HD + 1, 512], f32, tag="cA")
                ctxB = cpB.tile([HD + 1, 512], f32, tag="cB")
                for c in range(16):
                    sps = sp.tile([128, 1024], f32, tag="s")
                    mm(
                        sps[:, 0:512],
                        k_sb[0:64, p, ds(c * 128, 128)],
                        q_sb[0:64, p, qsl],
                        start=True,
                        stop=True,
                        tile_position=(0, 0),
                    )
                    mm(
                        sps[:, 512:1024],
                        k_sb[64:128, p, ds(c * 128, 128)],
                        q_sb[64:128, p, qsl],
                        start=True,
                        stop=True,
                        tile_position=(64, 0),
                    )
                    et = ep.tile([128, 1024], bf16, tag="e")
                    nc.scalar.activation(
                        et[:], sps[:], EXP, bias=mask_sb[:, c : c + 1], scale=0.125
                    )
                    mm(
                        ctxA[:],
                        v_sb[:, c, hA, :],
                        et[:, 0:512],
                        start=(c == 0),
                        stop=(c == 15),
                    )
                    mm(
                        ctxB[:],
                        v_sb[:, c, hB, :],
                        et[:, 512:1024],
                        start=(c == 0),
                        stop=(c == 15),
                    )
                for h, ctx in ((hA, ctxA), (hB, ctxB)):
                    # stage out of PSUM fast so the next qq's PV can start;
                    # the normalize chain then runs off the critical path
                    stage = fp.tile([HD + 1, 512], f32, tag="stage")
                    nc.vector.tensor_copy(stage[:], ctx[:])
                    recip = fp.tile([1, 512], f32, tag="recip")
                    nc.vector.reciprocal(recip[:], stage[HD : HD + 1, :])
                    bcast = fp.tile([64, 512], f32, tag="bcast")
                    nc.gpsimd.partition_broadcast(bcast[:], recip[:])
                    out_sb = fp.tile([64, 512], f32, tag="out")
                    nc.vector.tensor_mul(out_sb[:], stage[0:HD, :], bcast[:])
                    rp = (h % 2) * 64
                    nc.vector.tensor_scalar_add(
                        out_sb[:], out_sb[:], bv_sb[rp : rp + 64, h // 2 : h // 2 + 1]
                    )
                    nc.sync.dma_start(out_d[ds(h * 64, 64), qsl], out_sb[:])

            # ---- pass A: K(i=0), V(all), Q(j=0), streaming X^T ----
            for n in range(4):
                if n == 0:
                    x_n = x0
                else:
                    x_n = xpool.tile([128, 8, 512], bf16, tag="x")
                    nc.sync.dma_start(
                        x_n[:],
                        x_d[ds(n * DM, DM), :].rearrange("(c p) s -> p c s", p=128),
                    )
                qk_proj(wkt, bk_sb, k_sb, 0, n, x_n)
                for m in range(4):
                    v_proj(m, n, x_n)
                qk_proj(wqt, bq_sb, q_sb, 0, n, x_n)

            # ---- attention pair p overlapped with projections for p+1 ----
            for p in range(4):
                if p > 0:
                    wkt = load_w_tile(wk_d, p)
                    wqt = load_w_tile(wq_d, p)
                    for n in range(4):
                        x_n = xpool.tile([128, 8, 512], bf16, tag="x")
                        nc.sync.dma_start(
                            x_n[:],
                            x_d[ds(n * DM, DM), :].rearrange("(c p) s -> p c s", p=128),
                        )
                        qk_proj(wkt, bk_sb, k_sb, p, n, x_n)
                        qk_proj(wqt, bq_sb, q_sb, p, n, x_n)
                for qq in range(4):
                    attn_pair(p, qq)

    nc.compile()
    return nc


def make_in_maps(
    hidden_states, attention_mask, q_weight, q_bias, k_weight, k_bias, v_weight, v_bias
):
    hs = np.asarray(hidden_states, dtype=np.float32)
    am = np.asarray(attention_mask, dtype=np.float32)
    ws = {
        "q": np.asarray(q_weight, dtype=np.float32),
        "k": np.asarray(k_weight, dtype=np.float32),
        "v": np.asarray(v_weight, dtype=np.float32),
    }
    bs = {
        "q": np.asarray(q_bias, dtype=np.float32),
        "k": np.asarray(k_bias, dtype=np.float32),
        "v": np.asarray(v_bias, dtype=np.float32),
    }
    in_maps = []
    for core in range(NC):
        b, half = divmod(core, 2)
        fsl = slice(half * F, (half + 1) * F)
        in_maps.append(
            {
                "x_t": np.ascontiguousarray(
                    hs[b].T.reshape(DM, 4, 512).transpose(1, 0, 2).reshape(4 * DM, 512)
                ).astype(_bf16np),
                "wq_t": np.ascontiguousarray(
                    ws["q"][fsl, :].T.reshape(DM, 4, 128).transpose(1, 0, 2).reshape(4 * DM, 128)
                ).astype(_bf16np),
                "wk_t": np.ascontiguousarray(
                    ws["k"][fsl, :].T.reshape(DM, 4, 128).transpose(1, 0, 2).reshape(4 * DM, 128)
                ).astype(_bf16np),
                "wv_t": np.ascontiguousarray(ws["v"][fsl, :].T).astype(_bf16np),
                "bq": np.ascontiguousarray(bs["q"][fsl]).reshape(F, 1),
                "bk": np.ascontiguousarray(bs["k"][fsl]).reshape(F, 1),
                "bv": np.ascontiguousarray(bs["v"][fsl]).reshape(F, 1),
                "mask": np.ascontiguousarray(am[b, 0, 0, :].reshape(16, 128).T),
            }
        )
    return in_maps


def assemble_out(results):
    out = np.empty((4, S, DM), dtype=np.float32)
    for core in range(NC):
        b, half = divmod(core, 2)
        out[b, :, half * F : (half + 1) * F] = results[core]["out_t"].T
    return out


_NC_CACHE = []


def _run(inputs, trace=False):
    from concourse.bass_utils import run_bass_kernel_spmd

    if not _NC_CACHE:
        _NC_CACHE.append(build_nc())
    nc = _NC_CACHE[0]
    in_maps = make_in_maps(**inputs)
    res = run_bass_kernel_spmd(nc, in_maps, list(range(NC)), trace=trace)
    return assemble_out(res.results), res


def kernel(**inputs):
    out, _ = _run(inputs, trace=False)
    return out
